# revision 17
# baseline (speedup 1.0000x reference)
"""GPT forward pass on 8 Trainium2 NeuronCores (Bass/Tile).

Sharding: sequence-parallel over tokens (core c owns 256 tokens: seq c//4,
block c%4) with head-sharded attention inside each 4-core sequence group.
Per layer: AllGather of post-LN1 activations (group of 4), local QKV for the
core's 4 heads over the whole sequence, causal attention, AllToAll (8-core
mesh, duplicated blocks + data-driven mask select) to return to token
sharding, then local Wo/FFN on the core's own 256 tokens with replicated
weights.  Final LN + 8-core AllGather + vocab-parallel output projection
(4000 vocab columns per core).

Activations feature-major [D, tokens]; residual stream fp32; matmul operands
bf16; layernorm statistics via ones-matmul column sums on the PE.
"""

import os
import sys

import numpy as np

for _p in (
    "/opt/trn_rl_repo",
    "/root/.axon_site",
    "/root/.axon_site/_ro/trn_rl_repo",
    "/root/.axon_site/_ro/pypackages",
):
    if os.path.isdir(_p) and _p not in sys.path:
        sys.path.append(_p)

import ml_dtypes  # noqa: E402
import concourse.bacc as bacc  # noqa: E402
import concourse.tile as tile  # noqa: E402
import concourse.mybir as mybir  # noqa: E402
from concourse import bass_utils  # noqa: E402

BF16 = mybir.dt.bfloat16
F32 = mybir.dt.float32
AF = mybir.ActivationFunctionType
AO = mybir.AluOpType
BF16_NP = ml_dtypes.bfloat16

V, L, D, NB, H, HD = 32000, 1024, 1024, 6, 16, 64
B, S = 2, 1024
FF = 4 * D
EPS = 1e-5
N_CORES = 8
G = 4                 # cores per sequence group
TOK = 256             # tokens owned per core
KC = D // 128         # feature chunks of 128
HPC = H // G          # heads per core
VS = V // N_CORES     # vocab shard per core
QT = S // 128         # query tiles per sequence


def _build(nb=NB, stage=99):
    nc = bacc.Bacc("TRN2", target_bir_lowering=False, debug=False,
                   num_devices=N_CORES)

    def din(name, shape, dt):
        return nc.dram_tensor(name, shape, dt, kind="ExternalInput").ap()

    x0T = din("x0T", [D, TOK], F32)
    qkv_w = din("qkv_w", [nb, D, 3 * TOK], BF16)
    wo_w = din("wo_w", [nb, D, D], BF16)
    w1_w = din("w1_w", [nb, D, FF], BF16)
    w2_w = din("w2_w", [nb, FF, D], BF16)
    wp_w = din("wp_w", [D, VS], BF16)
    bp_b = din("bp_b", [1, VS], BF16)
    ln1_g = din("ln1_g", [nb, D], F32)
    ln1_b = din("ln1_b", [nb, D], F32)
    ln2_g = din("ln2_g", [nb, D], F32)
    ln2_b = din("ln2_b", [nb, D], F32)
    lnf_g = din("lnf_g", [1, D], F32)
    lnf_b = din("lnf_b", [1, D], F32)
    bo_b = din("bo_b", [nb, D], F32)
    b1_b = din("b1_b", [nb, FF], F32)
    b2_b = din("b2_b", [nb, D], F32)
    tri_in = din("tri", [128, HPC * 128], BF16)
    m0_in = din("m0", [128, 1], F32)
    m1_in = din("m1", [128, 1], F32)

    logits = nc.dram_tensor("logits", [B * S, VS], F32, kind="ExternalOutput").ap()

    groups4 = [[0, 1, 2, 3], [4, 5, 6, 7]]
    groups8 = [list(range(N_CORES))]

    from contextlib import ExitStack
    with tile.TileContext(nc) as tc, ExitStack() as es:
        consts = es.enter_context(tc.tile_pool(name="consts", bufs=1))
        params_pool = es.enter_context(tc.tile_pool(name="params", bufs=2))
        wqkv_pool = es.enter_context(tc.tile_pool(name="wqkv", bufs=3))
        wo_pool = es.enter_context(tc.tile_pool(name="wop", bufs=3))
        wbig_pool = es.enter_context(tc.tile_pool(name="wbig", bufs=2))
        act_pool = es.enter_context(tc.tile_pool(name="acts", bufs=2))
        big_act_pool = es.enter_context(tc.tile_pool(name="bigacts", bufs=1))
        small_pool = es.enter_context(tc.tile_pool(name="small", bufs=2))
        norm_pool = es.enter_context(tc.tile_pool(name="norm", bufs=2))
        probs_pool = es.enter_context(tc.tile_pool(name="probs", bufs=3))
        out_pool = es.enter_context(tc.tile_pool(name="outs", bufs=2))
        ps_pool = es.enter_context(tc.tile_pool(name="ps", bufs=3, space="PSUM"))
        ps_acc_pool = es.enter_context(tc.tile_pool(name="psacc", bufs=2, space="PSUM"))
        ps_sm_pool = es.enter_context(tc.tile_pool(name="pssm", bufs=2, space="PSUM"))
        dram = es.enter_context(tc.tile_pool(name="dram", bufs=2, space="DRAM"))

        # --- constants ---
        ones_stat = consts.tile([128, 1], F32)
        nc.vector.memset(ones_stat[:], 1.0)
        ones_bc = consts.tile([1, 128], F32)
        nc.vector.memset(ones_bc[:], 1.0)
        ones_bcb = consts.tile([1, 128], BF16)
        nc.vector.memset(ones_bcb[:], 1.0)
        tri_sb = consts.tile([128, HPC * 128], BF16)
        nc.sync.dma_start(tri_sb[:], tri_in[:])
        m0_sb = consts.tile([128, 1], F32)
        nc.sync.dma_start(m0_sb[:], m0_in[:])
        m1_sb = consts.tile([128, 1], F32)
        nc.sync.dma_start(m1_sb[:], m1_in[:])
        bp_sb = consts.tile([1, VS], BF16)
        nc.sync.dma_start(bp_sb[:], bp_b[:])
        lnfg_sb = consts.tile([128, KC], F32)
        nc.sync.dma_start(lnfg_sb[:], lnf_g[0, :].rearrange("(kc p) -> p kc", p=128))
        lnfb_sb = consts.tile([128, KC], F32)
        nc.sync.dma_start(lnfb_sb[:], lnf_b[0, :].rearrange("(kc p) -> p kc", p=128))
        eps_sb = consts.tile([128, 1], F32)
        nc.vector.memset(eps_sb[:], EPS)

        # --- residual stream (feature-major, fp32), persists across layers ---
        xT = consts.tile([128, KC, TOK], F32)
        nc.sync.dma_start(xT[:], x0T.rearrange("(kc p) t -> p kc t", p=128))

        def emit_ln(g_sb, b_sb, h_t):
            """h_t[:, kc, :] = LN(xT)[:, kc, :] (bf16), feature-major."""
            sum_ps = ps_sm_pool.tile([1, TOK], F32, name="lnsum", tag="stat")
            sq_ps = ps_sm_pool.tile([1, TOK], F32, name="lnsq", tag="stat")
            for kc in range(KC):
                xk = xT[:, kc, :]
                nc.tensor.matmul(sum_ps[:], ones_stat[:], xk,
                                 start=(kc == 0), stop=(kc == KC - 1))
                xsq = norm_pool.tile([128, TOK], F32, name="xsq")
                nc.vector.tensor_mul(xsq[:], xk, xk)
                nc.tensor.matmul(sq_ps[:], ones_stat[:], xsq[:],
                                 start=(kc == 0), stop=(kc == KC - 1))
            muden = small_pool.tile([1, 2 * TOK], F32, name="muden")
            nc.vector.tensor_scalar_mul(muden[:, 0:TOK], sum_ps[:], 1.0 / D)
            var_t = small_pool.tile([1, TOK], F32, name="var")
            nc.vector.tensor_scalar_mul(var_t[:], sq_ps[:], 1.0 / D)
            musq = small_pool.tile([1, TOK], F32, name="musq")
            nc.vector.tensor_mul(musq[:], muden[:, 0:TOK], muden[:, 0:TOK])
            nc.vector.tensor_sub(var_t[:], var_t[:], musq[:])
            logv = small_pool.tile([1, TOK], F32, name="logv")
            nc.scalar.activation(logv[:], var_t[:], AF.Ln, bias=eps_sb[0:1, :])
            # rstd = exp(-0.5 * log(var + eps))
            nc.scalar.activation(muden[:, TOK:2 * TOK], logv[:], AF.Exp, scale=-0.5)
            bc_ps = ps_sm_pool.tile([128, 2 * TOK], F32, name="lnbc", tag="bc", bufs=1)
            nc.tensor.matmul(bc_ps[:], ones_bc[:], muden[:], start=True, stop=True)
            for kc in range(KC):
                t = norm_pool.tile([128, TOK], F32, name="lnt")
                nc.vector.tensor_sub(t[:], xT[:, kc, :], bc_ps[:, 0:TOK])
                nc.vector.tensor_mul(t[:], t[:], bc_ps[:, TOK:2 * TOK])
                nc.vector.tensor_scalar(h_t[:, kc, :], t[:],
                                        g_sb[:, kc:kc + 1], b_sb[:, kc:kc + 1],
                                        AO.mult, AO.add)

        for l in range(nb):
            # --- per-layer params ---
            l1g = params_pool.tile([128, KC], F32, name="l1g")
            nc.sync.dma_start(l1g[:], ln1_g[l].rearrange("(kc p) -> p kc", p=128))
            l1b = params_pool.tile([128, KC], F32, name="l1b")
            nc.sync.dma_start(l1b[:], ln1_b[l].rearrange("(kc p) -> p kc", p=128))
            l2g = params_pool.tile([128, KC], F32, name="l2g")
            nc.sync.dma_start(l2g[:], ln2_g[l].rearrange("(kc p) -> p kc", p=128))
            l2b = params_pool.tile([128, KC], F32, name="l2b")
            nc.sync.dma_start(l2b[:], ln2_b[l].rearrange("(kc p) -> p kc", p=128))
            bo_sb = params_pool.tile([128, KC], F32, name="bo")
            nc.sync.dma_start(bo_sb[:], bo_b[l].rearrange("(kc p) -> p kc", p=128))
            b1_sb = params_pool.tile([128, FF // 128], F32, name="b1")
            nc.sync.dma_start(b1_sb[:], b1_b[l].rearrange("(m p) -> p m", p=128))
            b2_sb = params_pool.tile([128, KC], F32, name="b2")
            nc.sync.dma_start(b2_sb[:], b2_b[l].rearrange("(kc p) -> p kc", p=128))

            # --- weights (streamed in quarters, consumed sequentially) ---
            qkv_q = []
            for ww in range(3):
                t = wqkv_pool.tile([128, KC, TOK], BF16, name=f"qkvw{ww}", tag="qkv")
                nc.sync.dma_start(
                    t[:], qkv_w[l, :, ww * TOK:(ww + 1) * TOK]
                    .rearrange("(kc p) m -> p kc m", p=128))
                qkv_q.append(t)
            wo_q = []
            for qq in range(4):
                t = wo_pool.tile([128, KC, D // 4], BF16, name=f"wo{qq}", tag="wo")
                nc.sync.dma_start(
                    t[:], wo_w[l, :, qq * (D // 4):(qq + 1) * (D // 4)]
                    .rearrange("(kc p) m -> p kc m", p=128))
                wo_q.append(t)
            w1_q = []
            for qq in range(4):
                t = wbig_pool.tile([128, KC, FF // 4], BF16, name=f"w1{qq}", tag="wbig")
                nc.sync.dma_start(
                    t[:], w1_w[l, :, qq * (FF // 4):(qq + 1) * (FF // 4)]
                    .rearrange("(kc p) m -> p kc m", p=128))
                w1_q.append(t)
            w2_q = []
            for qq in range(4):
                t = wbig_pool.tile([128, FF // 128, D // 4], BF16, name=f"w2{qq}", tag="wbig")
                nc.sync.dma_start(
                    t[:], w2_w[l, :, qq * (D // 4):(qq + 1) * (D // 4)]
                    .rearrange("(kc p) m -> p kc m", p=128))
                w2_q.append(t)

            # --- LN1 ---
            h_t = act_pool.tile([128, KC, TOK], BF16, name="h1")
            emit_ln(l1g, l1b, h_t)
            if stage < 2:
                break

            # --- AllGather h across the sequence group ---
            hag_in = dram.tile([D, TOK], BF16, name="hag_in")
            nc.sync.dma_start(hag_in.rearrange("(kc p) t -> p kc t", p=128), h_t[:])
            hag_out = dram.tile([G * D, TOK], BF16, name="hag_out")
            nc.gpsimd.collective_compute(
                "AllGather", AO.bypass, replica_groups=groups4,
                ins=[hag_in[:]], outs=[hag_out[:]])
            hTf = act_pool.tile([128, KC, G, TOK], BF16, name="hTf", bufs=1)
            for rr in range(G):
                nc.sync.dma_start(
                    hTf[:, :, rr, :],
                    hag_out[rr * D:(rr + 1) * D, :].rearrange("(kc p) t -> p kc t", p=128))
            if stage < 3:
                break

            # --- QKV for my 4 heads over the whole sequence ---
            qT_t = act_pool.tile([128, 2, S], BF16, name="qT", bufs=1)
            kT_t = act_pool.tile([128, 2, S], BF16, name="kT", bufs=1)
            for dst, wsb in ((qT_t, qkv_q[0]), (kT_t, qkv_q[1])):
                for m in range(2):
                    for nn in range(2):
                        ps = ps_pool.tile([128, 512], F32, name="pjps", tag="mm")
                        for kc in range(KC):
                            nc.tensor.matmul(
                                ps[:],
                                wsb[:, kc, m * 128:(m + 1) * 128],
                                hTf[:, kc, nn * 2:nn * 2 + 2, :],
                                start=(kc == 0), stop=(kc == KC - 1))
                        nc.vector.tensor_copy(dst[:, m, nn * 512:(nn + 1) * 512], ps[:])
            v_aug = act_pool.tile([128, QT, HPC, HD + 1], BF16, name="vaug", bufs=1)
            nc.vector.memset(v_aug[:, :, :, HD:HD + 1], 1.0)
            for tt in range(QT):
                ps = ps_pool.tile([128, TOK], F32, name="pvps", tag="mm")
                for kc in range(KC):
                    nc.tensor.matmul(
                        ps[:],
                        hTf[:, kc, tt // 2, (tt % 2) * 128:(tt % 2) * 128 + 128],
                        qkv_q[2][:, kc, :],
                        start=(kc == 0), stop=(kc == KC - 1))
                nc.vector.tensor_copy(v_aug[:, tt, :, 0:HD],
                                      ps[:].rearrange("p (h d) -> p h d", h=HPC))

            if stage < 4:
                break
            # --- causal attention for my 4 heads ---
            a2a_in = dram.tile([2 * S, TOK], BF16, name="a2a_in")
            for j in range(QT):
                attn_ps = ps_acc_pool.tile([128, HPC * 128], F32, name="attnps")
                ORDER = [0, 2, 1, 3]  # probs col-block b holds head ORDER[b]
                for i in range(j + 1):
                    # two PSUM banks: one per PE row-group (sub 0 / sub 1) —
                    # different row-groups writing one bank crashes the HW
                    sc_a = ps_pool.tile([128, 256], F32, name="scpsa", tag="mm")
                    sc_b = ps_pool.tile([128, 256], F32, name="scpsb", tag="mm")
                    for h in range(HPC):
                        sub, pr = h % 2, h // 2
                        dst_ps = sc_a if sub == 0 else sc_b
                        nc.tensor.matmul(
                            dst_ps[:, pr * 128:(pr + 1) * 128],
                            kT_t[sub * 64:(sub + 1) * 64, pr, i * 128:(i + 1) * 128],
                            qT_t[sub * 64:(sub + 1) * 64, pr, j * 128:(j + 1) * 128],
                            start=True, stop=True)
                    probs = probs_pool.tile([128, HPC * 128], BF16, name="probs")
                    nc.scalar.activation(probs[:, 0:256], sc_a[:], AF.Exp,
                                         scale=1.0 / np.sqrt(HD))
                    nc.scalar.activation(probs[:, 256:512], sc_b[:], AF.Exp,
                                         scale=1.0 / np.sqrt(HD))
                    if i == j:
                        nc.vector.tensor_mul(probs[:], probs[:], tri_sb[:])
                    for b in range(HPC):
                        nc.tensor.matmul(
                            attn_ps[0:HD + 1, b * 128:(b + 1) * 128],
                            v_aug[:, i, ORDER[b], :],
                            probs[:, b * 128:(b + 1) * 128],
                            start=(i == 0 and b == 0),
                            stop=(i == j and b == HPC - 1))
                rden = small_pool.tile([1, HPC * 128], F32, name="rden")
                nc.vector.reciprocal(rden[:], attn_ps[HD:HD + 1, :])
                rbc_ps = ps_sm_pool.tile([64, HPC * 128], F32, name="rbc", tag="bc", bufs=1)
                nc.tensor.matmul(rbc_ps[:], ones_bc[0:1, 0:64], rden[:],
                                 start=True, stop=True)
                rbc_sb = small_pool.tile([64, HPC * 128], F32, name="rbcsb")
                nc.vector.tensor_copy(rbc_sb[:], rbc_ps[:])
                attn_sb = out_pool.tile([64, HPC * 128], BF16, name="attnsb")
                nc.vector.tensor_mul(attn_sb[:], attn_ps[0:HD, :], rbc_sb[:])
                # scatter into the A2A buffer (both group halves); attn_sb
                # col-block b holds head ORDER[b] = 2*pr + sb with b = 2*sb + pr
                for half in range(2):
                    base = half * S + (j // 2) * TOK
                    dst = a2a_in[base:base + TOK, :].rearrange(
                        "(pr sb d) t -> sb d pr t", pr=2, sb=2)
                    for sb in range(2):
                        nc.sync.dma_start(
                            dst[sb, :, :, (j % 2) * 128:(j % 2) * 128 + 128],
                            attn_sb[:, sb * 256:(sb + 1) * 256]
                            .rearrange("d (pr q) -> d pr q", pr=2))

            if stage < 5:
                break
            a2a_out = dram.tile([2 * S, TOK], BF16, name="a2a_out")
            nc.gpsimd.collective_compute(
                "AllToAll", AO.bypass, replica_groups=groups8,
                ins=[a2a_in[:]], outs=[a2a_out[:]])
            tA = act_pool.tile([128, KC, TOK], BF16, name="tA", bufs=1)
            nc.sync.dma_start(tA[:], a2a_out[0:D, :].rearrange("(kc p) t -> p kc t", p=128))
            tB = act_pool.tile([128, KC, TOK], BF16, name="tB", bufs=1)
            nc.sync.dma_start(tB[:], a2a_out[D:2 * D, :]
                              .rearrange("(kc p) t -> p kc t", p=128))
            attnT = act_pool.tile([128, KC, TOK], BF16, name="attnT", bufs=1)
            nc.vector.tensor_scalar_mul(tA[:], tA[:], m0_sb[:])
            nc.vector.tensor_scalar_mul(tB[:], tB[:], m1_sb[:])
            nc.vector.tensor_add(attnT[:], tA[:], tB[:])

            if stage < 6:
                break
            # --- Wo projection + residual ---
            for mc in range(KC):
                ps = ps_pool.tile([128, TOK], F32, name="wops", tag="mm")
                wo_sb = wo_q[mc // 2]
                mloc = mc % 2
                for kc in range(KC):
                    nc.tensor.matmul(ps[:],
                                     wo_sb[:, kc, mloc * 128:(mloc + 1) * 128],
                                     attnT[:, kc, :],
                                     start=(kc == 0), stop=(kc == KC - 1))
                t = norm_pool.tile([128, TOK], F32, name="rest")
                nc.vector.tensor_scalar_add(t[:], ps[:], bo_sb[:, mc:mc + 1])
                nc.vector.tensor_add(xT[:, mc, :], xT[:, mc, :], t[:])

            if stage < 7:
                break
            # --- FFN ---
            h2_t = act_pool.tile([128, KC, TOK], BF16, name="h2")
            emit_ln(l2g, l2b, h2_t)
            ff_t = big_act_pool.tile([128, FF // 128, TOK], BF16, name="ff", tag="ff")
            for m in range(FF // 128):
                ps = ps_pool.tile([128, TOK], F32, name="f1ps", tag="mm")
                w1_sb = w1_q[m // 8]
                mloc = m % 8
                for kc in range(KC):
                    nc.tensor.matmul(ps[:],
                                     w1_sb[:, kc, mloc * 128:(mloc + 1) * 128],
                                     h2_t[:, kc, :],
                                     start=(kc == 0), stop=(kc == KC - 1))
                nc.scalar.activation(ff_t[:, m, :], ps[:], AF.Relu,
                                     bias=b1_sb[:, m:m + 1])
            for mc in range(KC):
                ps = ps_pool.tile([128, TOK], F32, name="f2ps", tag="mm")
                w2_sb = w2_q[mc // 2]
                mloc = mc % 2
                for kc in range(FF // 128):
                    nc.tensor.matmul(ps[:],
                                     w2_sb[:, kc, mloc * 128:(mloc + 1) * 128],
                                     ff_t[:, kc, :],
                                     start=(kc == 0), stop=(kc == FF // 128 - 1))
                t = norm_pool.tile([128, TOK], F32, name="rest2")
                nc.vector.tensor_scalar_add(t[:], ps[:], b2_sb[:, mc:mc + 1])
                nc.vector.tensor_add(xT[:, mc, :], xT[:, mc, :], t[:])

        # --- final LN + AllGather + vocab projection ---
        if stage >= 8:
            xf_t = act_pool.tile([128, KC, TOK], BF16, name="xf")
            emit_ln(lnfg_sb, lnfb_sb, xf_t)
            xfag_in = dram.tile([D, TOK], BF16, name="xfag_in")
            nc.sync.dma_start(xfag_in.rearrange("(kc p) t -> p kc t", p=128), xf_t[:])
            xfag_out = dram.tile([N_CORES * D, TOK], BF16, name="xfag_out",
                                 addr_space="Shared")
            nc.gpsimd.collective_compute(
                "AllGather", AO.bypass, replica_groups=groups8,
                ins=[xfag_in[:]], outs=[xfag_out[:]])
            xf_all = big_act_pool.tile([128, KC, N_CORES, TOK], BF16, name="xfall",
                                       tag="ff")
            for rb in range(N_CORES):
                nc.sync.dma_start(
                    xf_all[:, :, rb, :],
                    xfag_out[rb * D:(rb + 1) * D, :]
                    .rearrange("(kc p) t -> p kc t", p=128))

            VC = 500
            for qq in range(4):
                wp_sb = wbig_pool.tile([128, KC, VS // 4], BF16, name=f"wp{qq}",
                                       tag="wbig")
                nc.sync.dma_start(
                    wp_sb[:], wp_w[:, qq * (VS // 4):(qq + 1) * (VS // 4)]
                    .rearrange("(kc p) m -> p kc m", p=128))
                for vc in range(2):
                    off = qq * (VS // 4) + vc * VC
                    for m in range(B * S // 128):
                        ps = ps_pool.tile([128, VC], F32, name="lps", tag="mm")
                        for kc in range(KC):
                            nc.tensor.matmul(
                                ps[:],
                                xf_all[:, kc, m // 2,
                                       (m % 2) * 128:(m % 2) * 128 + 128],
                                wp_sb[:, kc, vc * VC:(vc + 1) * VC],
                                start=(kc == 0), stop=False)
                        nc.tensor.matmul(ps[:], ones_bcb[:],
                                         bp_sb[:, off:off + VC],
                                         start=False, stop=True)
                        out_sb = out_pool.tile([128, VC], F32, name="lout")
                        nc.vector.tensor_copy(out_sb[:], ps[:])
                        nc.sync.dma_start(
                            logits[m * 128:(m + 1) * 128, off:off + VC],
                            out_sb[:])

    nc.compile()
    return nc


_CACHE = {}


def _built(nb=NB, stage=99):
    key = f"nc{nb}s{stage}"
    if key not in _CACHE:
        _CACHE[key] = _build(nb, stage)
    return _CACHE[key]


def make_in_maps(context, params, nb=NB):
    """Host-side sharding: returns in_maps list for the 8 cores."""
    context = np.asarray(context)
    tok_emb = np.asarray(params["tok_emb"], np.float32)
    pos_emb = np.asarray(params["pos_emb"], np.float32)
    x0 = tok_emb[context] + pos_emb[None, :S, :]          # [B, S, D] fp32

    def bf(x):
        return np.ascontiguousarray(np.asarray(x)).astype(BF16_NP)

    wq = np.asarray(params["Wq"], np.float32)[:nb]
    wk = np.asarray(params["Wk"], np.float32)[:nb]
    wv = np.asarray(params["Wv"], np.float32)[:nb]
    qkv_by_rank = []
    for r in range(G):
        sl = slice(r * TOK, (r + 1) * TOK)
        qkv_by_rank.append(bf(np.concatenate(
            [wq[:, :, sl], wk[:, :, sl], wv[:, :, sl]], axis=2)))
    wo = bf(params["Wo"])[:nb]
    w1 = bf(params["W1"])[:nb]
    w2 = bf(params["W2"])[:nb]
    wp = np.asarray(params["Wp"], np.float32)
    bp = np.asarray(params["bp"], np.float32)

    f32 = lambda k: np.ascontiguousarray(np.asarray(params[k], np.float32))[:nb]
    tri = np.tile(np.triu(np.ones((128, 128), np.float32)), (1, HPC)).astype(BF16_NP)

    in_maps = []
    for c in range(N_CORES):
        seq, r = c // G, c % G
        x0T_c = np.ascontiguousarray(x0[seq, r * TOK:(r + 1) * TOK, :].T)
        m0 = np.full((128, 1), 1.0 if c < G else 0.0, np.float32)
        m1 = np.full((128, 1), 0.0 if c < G else 1.0, np.float32)
        in_maps.append({
            "x0T": x0T_c,
            "qkv_w": qkv_by_rank[r],
            "wo_w": wo, "w1_w": w1, "w2_w": w2,
            "wp_w": bf(wp[:, c * VS:(c + 1) * VS]),
            "bp_b": bf(bp[c * VS:(c + 1) * VS])[None, :],
            "ln1_g": f32("ln1_g"), "ln1_b": f32("ln1_b"),
            "ln2_g": f32("ln2_g"), "ln2_b": f32("ln2_b"),
            "lnf_g": np.asarray(params["lnf_g"], np.float32)[None, :],
            "lnf_b": np.asarray(params["lnf_b"], np.float32)[None, :],
            "bo_b": f32("bo"), "b1_b": f32("b1"), "b2_b": f32("b2"),
            "tri": tri, "m0": m0, "m1": m1,
        })
    return in_maps


def run(context, params, trace=False, nb=NB, stage=99):
    nc = _built(nb, stage)
    in_maps = make_in_maps(context, params, nb)
    res = bass_utils.run_bass_kernel_spmd(
        nc, in_maps, core_ids=list(range(N_CORES)), trace=trace)
    out = np.concatenate([res.results[c]["logits"] for c in range(N_CORES)], axis=1)
    return out.reshape(B, S, V).astype(np.float32), res


def kernel(context, params):
    out, _ = run(context, params)
    return out
